# revision 34
# baseline (speedup 1.0000x reference)
"""Trainium2 Bass kernel for a per-token fake-quantized Linear:

    y = fake_quant(fake_quant(x) @ W.T + b)      (per-token int8 symmetric)

x: [4, 2048, 4096] f32, W: [4096, 4096] f32, b: [4096] f32.

Strategy (8 NeuronCores, pure data parallel over tokens - zero collectives):
  - 8192 tokens / 8 cores = 1024 tokens per core; W, b replicated.
  - Per-token quantized x values are integers in [-127, 127], EXACTLY
    representable in bf16, so the matmul runs on TensorE in bf16 with f32
    PSUM accumulation (W pre-packed to bf16 W.T on the host).
  - NATURAL-LAYOUT matmul: stationary operand = q^T tiles [128k, 128t],
    moving operand = W.T [128k, 512o] -> PSUM holds y[token, out] directly.
    No output transposes, no z^T DRAM staging: the per-token output absmax
    is a free-dim reduce folded into PSUM evacuation, and requant runs
    straight out of SBUF at the end of each token-half.
  - Bias: y = s_x*(z + rinv_x*b), applied during PSUM evacuation as ONE
    VectorE scalar_tensor_tensor op (u = b_block*rinv + psum) - no K=1
    bias matmuls on the critical PE stream.
  - Tokens processed in two halves of 512 so half A's requant/output DMA
    overlaps half B's matmuls; W is re-streamed per half (2x33.5 MB still
    well under the DMA roofline).
  - q^T for half A is built with PE transposes (PE is idle during pass 1),
    half B via DRAM-staged xbar transposes on the scalar HWDGE ring,
    overlapped with half A's matmuls.
  - Data-dependent DMAs (x half-B loads, q staging, output stores) ride
    the gpsimd SWDGE queue so they never head-of-line-block the W
    prefetch streams on the two HWDGE rings.
  - Rounding is exact round-to-nearest-even via +/-1.5*2^23 magic adds.
"""

import sys

if "/opt/trn_rl_repo" not in sys.path:
    sys.path.insert(0, "/opt/trn_rl_repo")

from contextlib import ExitStack

import ml_dtypes
import numpy as np

import concourse.bass as bass
import concourse.mybir as mybir
import concourse.tile as tile
from concourse import bacc
from concourse.bass import ds
from concourse.bass_utils import run_bass_kernel_spmd
from concourse.masks import make_identity

N_CORES = 8
P = 128
T = 1024          # tokens per core
K = 4096          # in features
O = 4096          # out features
TT = T // P       # 8 token tiles
KT = K // P       # 32 k tiles
TH = T // 2       # tokens per half (512)
TTH = TT // 2     # token tiles per half (4)
OC = 512          # outputs per o-chunk (one PSUM bank of f32)
NOC = O // OC     # 8 o-chunks
KB = 8            # k-subtiles per W block (1 MiB)
NKB = KT // KB    # 4 W blocks per o-chunk

Q_MAX = 127.0
EPS = 1e-5
MAGIC = 1.5 * 2**23  # f32 add/sub forces round-to-nearest-even to integer
INV_QMAX = float(np.float32(1.0) / np.float32(Q_MAX))

F32 = mybir.dt.float32
BF16 = mybir.dt.bfloat16


def build():
    nc = bacc.Bacc()
    x_ext = nc.declare_dram_parameter("x", [T, K], F32, isOutput=False)
    wt_ext = nc.declare_dram_parameter("wt", [K, O], BF16, isOutput=False)
    b_ext = nc.declare_dram_parameter("b", [O], F32, isOutput=False)
    out_ext = nc.declare_dram_parameter("out", [T, O], F32, isOutput=True)

    with tile.TileContext(nc) as tc, ExitStack() as ctx:
        singles = ctx.enter_context(tc.tile_pool(name="singles", bufs=1))
        xp = ctx.enter_context(tc.tile_pool(name="xp", bufs=4))
        qp = ctx.enter_context(tc.tile_pool(name="qp", bufs=2))
        qt_pool = ctx.enter_context(tc.tile_pool(name="qt", bufs=1))
        sxp = ctx.enter_context(tc.tile_pool(name="sxp", bufs=1))
        stat = ctx.enter_context(tc.tile_pool(name="stat", bufs=4))
        wp = ctx.enter_context(tc.tile_pool(name="wp", bufs=4))
        up = ctx.enter_context(tc.tile_pool(name="up", bufs=1))
        yp = ctx.enter_context(tc.tile_pool(name="yp", bufs=4))
        psum = ctx.enter_context(tc.tile_pool(name="psum", bufs=6, space="PSUM"))
        tpp = ctx.enter_context(tc.tile_pool(name="tpp", bufs=2, space="PSUM"))

        identity = singles.tile([P, P], BF16, tag="identity")
        make_identity(nc, identity)

        # bias replicated across partitions: [128, O] bf16 (emitted after
        # pass-1 half A so it doesn't sit ahead of the quant rounds in the
        # gpsimd queue; first use is the first evac at ~60us)
        b_row = singles.tile([1, O], BF16, tag="b_row")
        b_block = singles.tile([P, O], BF16, tag="b_block")

        # q^T strips, one per k-tile: [128k, 1024t] bf16
        qt_tiles = [
            qt_pool.tile([P, T], BF16, tag=f"qt{k}", name=f"qt{k}")
            for k in range(KT)
        ]
        # per-token input scale / inverse scale, named per t-tile
        sx_tiles = [sxp.tile([P, 1], F32, tag=f"sx{t}", name=f"sx{t}")
                    for t in range(TT)]
        rinv_tiles = [sxp.tile([P, 1], F32, tag=f"rinv{t}", name=f"rinv{t}")
                      for t in range(TT)]
        # per-(t-tile, o-chunk) partial |u| maxima
        amp_tiles = [sxp.tile([P, NOC], F32, tag=f"amp{t}", name=f"amp{t}")
                     for t in range(TT)]
        # u = z + rinv_x*b staged per t-tile of the current half, bf16
        u_tiles = [up.tile([P, O], BF16, tag=f"u{i}", name=f"u{i}")
                   for i in range(TTH)]

        KH = K // 2

        # ---- pass 1: per-token scales + integer quant + q^T build ----
        def pass1_load(t):
            # half A: both halves on ONE ring (avoids cross-ring arrival
            # skew), alternating rings per tile. Half B: SWDGE, so the 2 MB
            # inserts don't delay the W stream on the HWDGE rings; issued one
            # o-chunk ahead of the quant so the gpsimd FIFO (where a waiting
            # round op head-of-line-blocks later DGE work) still delivers
            # the data in time.
            if t < TTH:
                eng = nc.sync if t % 2 == 0 else nc.scalar
            else:
                eng = nc.gpsimd
            xh = []
            for i in range(2):
                x_half = xp.tile([P, KH], F32, tag="x_half")
                eng.dma_start(out=x_half, in_=x_ext[ds(t * P, P), ds(i * KH, KH)])
                xh.append(x_half)
            return xh

        def pass1_quant(t, xh):
            amh = stat.tile([P, 2], F32, tag="am_x")
            for i in range(2):
                nc.vector.tensor_reduce(
                    out=amh[:, i:i + 1], in_=xh[i], axis=mybir.AxisListType.X,
                    op=mybir.AluOpType.max, apply_absolute_value=True,
                )
            # stats chain pulled ahead in scheduler priority so it isn't
            # interleaved behind the next tile's big reduces
            with tc.high_priority(offset=60):
                am = stat.tile([P, 1], F32, tag="am_c")
                nc.vector.tensor_reduce(
                    out=am, in_=amh, axis=mybir.AxisListType.X,
                    op=mybir.AluOpType.max,
                )
                # s = max(absmax, EPS) * (1/127)
                nc.vector.tensor_scalar(
                    out=sx_tiles[t], in0=am, scalar1=EPS, scalar2=INV_QMAX,
                    op0=mybir.AluOpType.max, op1=mybir.AluOpType.mult,
                )
                nc.vector.reciprocal(out=rinv_tiles[t], in_=sx_tiles[t])
            for i in range(2):
                # r = x * rinv + MAGIC  (in place, gpsimd), q = r - MAGIC -> bf16
                nc.gpsimd.tensor_scalar(
                    out=xh[i], in0=xh[i], scalar1=rinv_tiles[t], scalar2=MAGIC,
                    op0=mybir.AluOpType.mult, op1=mybir.AluOpType.add,
                )
                q_half = qp.tile([P, KH], BF16, tag="q_half")
                nc.vector.tensor_scalar(
                    out=q_half, in0=xh[i], scalar1=MAGIC,
                    scalar2=None, op0=mybir.AluOpType.subtract,
                )
                # PE-transpose q into the q^T strips. Half A runs while PE is
                # otherwise idle; half B's transposes interleave into half A's
                # matmul stream (+35us PE, but NO xbar DMA: dma_start_transpose
                # serializes against copy-mode DMA and starves the W stream)
                for j in range(KT // 2):
                    k = i * (KT // 2) + j
                    tp = tpp.tile([P, P], BF16, tag="tp")
                    nc.tensor.transpose(tp, q_half[:, ds(j * P, P)], identity)
                    nc.scalar.copy(out=qt_tiles[k][:, ds(t * P, P)], in_=tp)

        # ALL half-A loads (and half-B's first) issue before any quant op:
        # the gpsimd/ring FIFOs see pure loads up front, so no DGE or
        # transfer ever queues behind a dep-waiting quant round
        h1_loads = {}
        xh_all = [pass1_load(t) for t in range(TTH)]
        h1_loads[TTH] = pass1_load(TTH)
        for t in range(TTH):
            pass1_quant(t, xh_all[t])

        # bias load lands here in the gpsimd queue: after the half-A quant
        # rounds, well before the first evac needs b_block
        nc.gpsimd.dma_start(out=b_row, in_=b_ext[:])  # gpsimd DMA casts f32->bf16
        nc.gpsimd.partition_broadcast(b_block, b_row)

        # ---- matmul + fused evac/requant, one token-half at a time ----
        def evac(h, oc, ps, i):
            t = h * TTH + i
            # u = b*rinv_x + z   (one VectorE op, PSUM -> SBUF bf16)
            nc.vector.scalar_tensor_tensor(
                out=u_tiles[i][:, ds(oc * OC, OC)],
                in0=b_block[:, ds(oc * OC, OC)],
                scalar=rinv_tiles[t],
                in1=ps[i],
                op0=mybir.AluOpType.mult,
                op1=mybir.AluOpType.add,
            )
            nc.vector.tensor_reduce(
                out=amp_tiles[t][:, oc:oc + 1],
                in_=u_tiles[i][:, ds(oc * OC, OC)],
                axis=mybir.AxisListType.X,
                op=mybir.AluOpType.max, apply_absolute_value=True,
            )

        def w_fetch(oc, kb):
            w_tile = wp.tile([P, KB, OC], BF16, tag="w_tile")
            w_eng = nc.sync if (oc * NKB + kb) % 2 == 0 else nc.scalar
            w_eng.dma_start(
                out=w_tile,
                in_=wt_ext[
                    ds(kb * KB * P, KB * P), ds(oc * OC, OC)
                ].rearrange("(s p) o -> p s o", p=P),
            )
            return w_tile

        def matmul_half(h):
            t0 = h * TTH
            for oc in range(NOC):
                ps = [psum.tile([P, OC], F32, tag="ps", name=f"ps_{h}_{oc}_{i}")
                      for i in range(TTH)]
                if h == 0 and oc == 0:
                    # first o-chunk runs TOKEN-OUTER: its matmul stream needs
                    # only one quantized t-tile at a time (t0 by ~28us, t3
                    # not until ~53us), hiding pass-1's pipeline latency.
                    # All 4 W blocks stay live, hence wp bufs=NKB.
                    w_tiles = [w_fetch(oc, kb) for kb in range(NKB)]
                    for i in range(TTH):
                        for kb in range(NKB):
                            for s in range(KB):
                                k = kb * KB + s
                                nc.tensor.matmul(
                                    ps[i],
                                    qt_tiles[k][:, ds((t0 + i) * P, P)],
                                    w_tiles[kb][:, s, :],
                                    start=(k == 0),
                                    stop=(k == KT - 1),
                                )
                        evac(h, oc, ps, i)
                else:
                    for kb in range(NKB):
                        w_tile = w_fetch(oc, kb)
                        for s in range(KB):
                            k = kb * KB + s
                            for i in range(TTH):
                                nc.tensor.matmul(
                                    ps[i],
                                    qt_tiles[k][:, ds((t0 + i) * P, P)],
                                    w_tile[:, s, :],
                                    start=(k == 0),
                                    stop=(k == KT - 1),
                                )
                    for i in range(TTH):
                        evac(h, oc, ps, i)
                if h == 0:
                    # half-B pass 1 interleaved into the o-chunk loop: loads
                    # run ~2 chunks ahead of their quant chain, and the quant
                    # (whose PE-transposes the scheduler slots into the PE
                    # stream near this priority point) trails 2 chunks behind,
                    # so a late x arrival can never head-of-line-stall the
                    # matmul stream.
                    t_load = TTH + 1 + oc
                    if t_load < TT:
                        h1_loads[t_load] = pass1_load(t_load)
                    t_q = TTH + oc - 2
                    if TTH <= t_q < TT:
                        pass1_quant(t_q, h1_loads.pop(t_q))

        def requant_half(h):
            t0 = h * TTH
            OH = O // 4
            for i in range(TTH):
                t = t0 + i
                am = stat.tile([P, 1], F32, tag="am_u")
                nc.vector.tensor_reduce(
                    out=am, in_=amp_tiles[t], axis=mybir.AxisListType.X,
                    op=mybir.AluOpType.max,
                )
                # s_y = max(s_x * absmax_u, EPS) * (1/127)
                sy = stat.tile([P, 1], F32, tag="sy")
                nc.vector.tensor_scalar(
                    out=sy, in0=am, scalar1=sx_tiles[t], scalar2=EPS,
                    op0=mybir.AluOpType.mult, op1=mybir.AluOpType.max,
                )
                nc.vector.tensor_scalar(
                    out=sy, in0=sy, scalar1=INV_QMAX, scalar2=None,
                    op0=mybir.AluOpType.mult,
                )
                rinvy = stat.tile([P, 1], F32, tag="rinv_y")
                nc.vector.reciprocal(out=rinvy, in_=sy)
                # f = s_x * rinv_y : y*rinv_y == u*f
                f = stat.tile([P, 1], F32, tag="f")
                nc.vector.tensor_scalar(
                    out=f, in0=rinvy, scalar1=sx_tiles[t], scalar2=None,
                    op0=mybir.AluOpType.mult,
                )
                for c in range(4):
                    y_half = yp.tile([P, OH], F32, tag="y_half")
                    # r = u*f + MAGIC (gpsimd), y_q = (r - MAGIC)*s_y (vector)
                    nc.gpsimd.tensor_scalar(
                        out=y_half, in0=u_tiles[i][:, ds(c * OH, OH)],
                        scalar1=f, scalar2=MAGIC,
                        op0=mybir.AluOpType.mult, op1=mybir.AluOpType.add,
                    )
                    nc.vector.tensor_scalar(
                        out=y_half, in0=y_half, scalar1=MAGIC, scalar2=sy,
                        op0=mybir.AluOpType.subtract, op1=mybir.AluOpType.mult,
                    )
                    # half-A stores ride SWDGE (W still streams on the HWDGE
                    # rings then); half-B stores split across the two HWDGE
                    # rings, which are idle once the last W block is in
                    if h == 0:
                        out_eng = nc.gpsimd
                    else:
                        out_eng = nc.sync if c % 2 == 0 else nc.scalar
                    out_eng.dma_start(
                        out=out_ext[ds(t * P, P), ds(c * OH, OH)], in_=y_half
                    )

        for h in range(2):
            matmul_half(h)
            requant_half(h)

    nc.compile()
    return nc


_NC_CACHE = None


def _get_nc():
    global _NC_CACHE
    if _NC_CACHE is None:
        _NC_CACHE = build()
    return _NC_CACHE


def _run(x, W, b, trace=False):
    nc = _get_nc()
    x2d = np.ascontiguousarray(np.asarray(x, dtype=np.float32).reshape(-1, K))
    wt = np.ascontiguousarray(np.asarray(W, dtype=np.float32).T).astype(
        ml_dtypes.bfloat16
    )
    bf = np.ascontiguousarray(np.asarray(b, dtype=np.float32))
    in_maps = [
        {"x": np.ascontiguousarray(x2d[i * T:(i + 1) * T]), "wt": wt, "b": bf}
        for i in range(N_CORES)
    ]
    res = run_bass_kernel_spmd(nc, in_maps, list(range(N_CORES)), trace=trace)
    out = np.concatenate([res.results[i]["out"] for i in range(N_CORES)], axis=0)
    return out, res


def kernel(x, W, b):
    out, _ = _run(x, W, b, trace=False)
    return out.reshape(np.asarray(x).shape[:-1] + (O,)).astype(np.float32)


# revision 38
# speedup vs baseline: 1.1097x; 1.1097x over previous
"""Trainium2 Bass kernel for a per-token fake-quantized Linear:

    y = fake_quant(fake_quant(x) @ W.T + b)      (per-token int8 symmetric)

x: [4, 2048, 4096] f32, W: [4096, 4096] f32, b: [4096] f32.

Strategy (8 NeuronCores, pure data parallel over tokens - zero collectives):
  - 8192 tokens / 8 cores = 1024 tokens per core; W, b replicated.
  - Per-token quantized x values are integers in [-127, 127], EXACTLY
    representable in bf16, so the matmul runs on TensorE in bf16 with f32
    PSUM accumulation (W pre-packed to bf16 W.T on the host).
  - NATURAL-LAYOUT matmul: stationary operand = q^T tiles [128k, 128t],
    moving operand = W.T [128k, 512o] -> PSUM holds y[token, out] directly.
    No output transposes, no z^T DRAM staging: the per-token output absmax
    is a free-dim reduce folded into PSUM evacuation, and requant runs
    straight out of SBUF at the end of each token-half.
  - Bias: y = s_x*(z + rinv_x*b), applied during PSUM evacuation as ONE
    VectorE scalar_tensor_tensor op (u = b_block*rinv + psum) - no K=1
    bias matmuls on the critical PE stream.
  - Tokens processed in two halves of 512 so half A's requant/output DMA
    overlaps half B's matmuls; W is re-streamed per half (2x33.5 MB still
    well under the DMA roofline).
  - ALL q^T strips are built with PE transposes: half A while the PE is
    otherwise idle in pass 1, half B interleaved into half A's matmul
    stream (~19us of PE). DMA xbar transposes were measurably worse:
    dma_start_transpose serializes against copy-mode DMA and starved the
    W stream for ~36us.
  - Data-dependent DMAs (x half-B loads, half-A output stores) ride the
    gpsimd SWDGE queue so they never head-of-line-block the W prefetch
    streams on the two HWDGE rings; half-B outputs take the HWDGE rings,
    which are idle by then.
  - Rounding is exact round-to-nearest-even via +/-1.5*2^23 magic adds.
  - Fragile-by-measurement notes: requant rounds MUST stay on gpsimd
    (scalar.activation there flips the whole schedule into an
    LDWEIGHTS-exposed mode, 219->263ns per matmul); same flip hit a
    token-outer first-o-chunk variant. Half-B pass-1 emission is
    interleaved one tile per o-chunk with loads one chunk ahead.
"""

import sys

if "/opt/trn_rl_repo" not in sys.path:
    sys.path.insert(0, "/opt/trn_rl_repo")

from contextlib import ExitStack

import ml_dtypes
import numpy as np

import concourse.bass as bass
import concourse.mybir as mybir
import concourse.tile as tile
from concourse import bacc
from concourse.bass import ds
from concourse.bass_utils import run_bass_kernel_spmd
from concourse.masks import make_identity

N_CORES = 8
P = 128
T = 1024          # tokens per core
K = 4096          # in features
O = 4096          # out features
TT = T // P       # 8 token tiles
KT = K // P       # 32 k tiles
TH = T // 2       # tokens per half (512)
TTH = TT // 2     # token tiles per half (4)
OC = 512          # outputs per o-chunk (one PSUM bank of f32)
NOC = O // OC     # 8 o-chunks
KB = 8            # k-subtiles per W block (1 MiB)
NKB = KT // KB    # 4 W blocks per o-chunk

Q_MAX = 127.0
EPS = 1e-5
MAGIC = 1.5 * 2**23  # f32 add/sub forces round-to-nearest-even to integer
INV_QMAX = float(np.float32(1.0) / np.float32(Q_MAX))

F32 = mybir.dt.float32
BF16 = mybir.dt.bfloat16


def build():
    nc = bacc.Bacc()
    x_ext = nc.declare_dram_parameter("x", [T, K], F32, isOutput=False)
    wt_ext = nc.declare_dram_parameter("wt", [K, O], BF16, isOutput=False)
    b_ext = nc.declare_dram_parameter("b", [O], F32, isOutput=False)
    out_ext = nc.declare_dram_parameter("out", [T, O], F32, isOutput=True)

    with tile.TileContext(nc) as tc, ExitStack() as ctx:
        singles = ctx.enter_context(tc.tile_pool(name="singles", bufs=1))
        xp = ctx.enter_context(tc.tile_pool(name="xp", bufs=4))
        qp = ctx.enter_context(tc.tile_pool(name="qp", bufs=2))
        qt_pool = ctx.enter_context(tc.tile_pool(name="qt", bufs=1))
        sxp = ctx.enter_context(tc.tile_pool(name="sxp", bufs=1))
        stat = ctx.enter_context(tc.tile_pool(name="stat", bufs=4))
        wp = ctx.enter_context(tc.tile_pool(name="wp", bufs=3))
        up = ctx.enter_context(tc.tile_pool(name="up", bufs=1))
        yp = ctx.enter_context(tc.tile_pool(name="yp", bufs=6))
        psum = ctx.enter_context(tc.tile_pool(name="psum", bufs=6, space="PSUM"))
        tpp = ctx.enter_context(tc.tile_pool(name="tpp", bufs=2, space="PSUM"))

        identity = singles.tile([P, P], BF16, tag="identity")
        make_identity(nc, identity)

        # bias replicated across partitions: [128, O] bf16 (emitted after
        # pass-1 half A so it doesn't sit ahead of the quant rounds in the
        # gpsimd queue; first use is the first evac at ~60us)
        b_row = singles.tile([1, O], BF16, tag="b_row")
        b_block = singles.tile([P, O], BF16, tag="b_block")

        # q^T strips, one per k-tile: [128k, 1024t] bf16
        qt_tiles = [
            qt_pool.tile([P, T], BF16, tag=f"qt{k}", name=f"qt{k}")
            for k in range(KT)
        ]
        # per-token input scale / inverse scale, named per t-tile
        sx_tiles = [sxp.tile([P, 1], F32, tag=f"sx{t}", name=f"sx{t}")
                    for t in range(TT)]
        rinv_tiles = [sxp.tile([P, 1], F32, tag=f"rinv{t}", name=f"rinv{t}")
                      for t in range(TT)]
        # per-(t-tile, o-chunk) partial |u| maxima
        amp_tiles = [sxp.tile([P, NOC], F32, tag=f"amp{t}", name=f"amp{t}")
                     for t in range(TT)]
        # u = z + rinv_x*b staged per t-tile of the current half, bf16
        u_tiles = [up.tile([P, O], BF16, tag=f"u{i}", name=f"u{i}")
                   for i in range(TTH)]

        KH = K // 2

        # ---- pass 1: per-token scales + integer quant + q^T build ----
        def pass1_load(t):
            # half A: both halves on ONE ring (avoids cross-ring arrival
            # skew), alternating rings per tile. Half B: SWDGE, so the 2 MB
            # inserts don't delay the W stream on the HWDGE rings; issued one
            # o-chunk ahead of the quant so the gpsimd FIFO (where a waiting
            # round op head-of-line-blocks later DGE work) still delivers
            # the data in time.
            if t < TTH:
                eng = nc.sync if t % 2 == 0 else nc.scalar
            else:
                eng = nc.gpsimd
            xh = []
            for i in range(2):
                x_half = xp.tile([P, KH], F32, tag="x_half")
                eng.dma_start(out=x_half, in_=x_ext[ds(t * P, P), ds(i * KH, KH)])
                xh.append(x_half)
            return xh

        def pass1_quant(t, xh):
            amh = stat.tile([P, 2], F32, tag="am_x")
            for i in range(2):
                nc.vector.tensor_reduce(
                    out=amh[:, i:i + 1], in_=xh[i], axis=mybir.AxisListType.X,
                    op=mybir.AluOpType.max, apply_absolute_value=True,
                )
            # stats chain pulled ahead in scheduler priority so it isn't
            # interleaved behind the next tile's big reduces
            with tc.high_priority(offset=60):
                am = stat.tile([P, 1], F32, tag="am_c")
                nc.vector.tensor_reduce(
                    out=am, in_=amh, axis=mybir.AxisListType.X,
                    op=mybir.AluOpType.max,
                )
                # s = max(absmax, EPS) * (1/127)
                nc.vector.tensor_scalar(
                    out=sx_tiles[t], in0=am, scalar1=EPS, scalar2=INV_QMAX,
                    op0=mybir.AluOpType.max, op1=mybir.AluOpType.mult,
                )
                nc.vector.reciprocal(out=rinv_tiles[t], in_=sx_tiles[t])
            for i in range(2):
                # r = x * rinv + MAGIC  (in place, gpsimd), q = r - MAGIC -> bf16
                nc.gpsimd.tensor_scalar(
                    out=xh[i], in0=xh[i], scalar1=rinv_tiles[t], scalar2=MAGIC,
                    op0=mybir.AluOpType.mult, op1=mybir.AluOpType.add,
                )
                q_half = qp.tile([P, KH], BF16, tag="q_half")
                nc.vector.tensor_scalar(
                    out=q_half, in0=xh[i], scalar1=MAGIC,
                    scalar2=None, op0=mybir.AluOpType.subtract,
                )
                # PE-transpose q into the q^T strips. Half A runs while PE is
                # otherwise idle; half B's transposes interleave into half A's
                # matmul stream (+35us PE, but NO xbar DMA: dma_start_transpose
                # serializes against copy-mode DMA and starves the W stream)
                for j in range(KT // 2):
                    k = i * (KT // 2) + j
                    tp = tpp.tile([P, P], BF16, tag="tp")
                    nc.tensor.transpose(tp, q_half[:, ds(j * P, P)], identity)
                    nc.scalar.copy(out=qt_tiles[k][:, ds(t * P, P)], in_=tp)

        h1_loads = {}
        for t in range(TTH):
            xh = pass1_load(t)
            if t == TTH - 2:
                # first half-B load issued HERE: its SWDGE descriptor-gen
                # lands ahead of t2/t3's rounds (and the bias broadcast) in
                # the gpsimd FIFO, so the data is resident well before its
                # quant chain comes up at the end of o-chunk 0
                h1_loads[TTH] = pass1_load(TTH)
            pass1_quant(t, xh)

        # bias load lands here in the gpsimd queue: after the half-A quant
        # rounds, well before the first evac needs b_block
        nc.gpsimd.dma_start(out=b_row, in_=b_ext[:])  # gpsimd DMA casts f32->bf16
        nc.gpsimd.partition_broadcast(b_block, b_row)

        # ---- matmul + fused evac/requant, one token-half at a time ----
        def evac(h, oc, ps, i):
            t = h * TTH + i
            # u = b*rinv_x + z   (one VectorE op, PSUM -> SBUF bf16)
            nc.vector.scalar_tensor_tensor(
                out=u_tiles[i][:, ds(oc * OC, OC)],
                in0=b_block[:, ds(oc * OC, OC)],
                scalar=rinv_tiles[t],
                in1=ps[i],
                op0=mybir.AluOpType.mult,
                op1=mybir.AluOpType.add,
            )
            nc.vector.tensor_reduce(
                out=amp_tiles[t][:, oc:oc + 1],
                in_=u_tiles[i][:, ds(oc * OC, OC)],
                axis=mybir.AxisListType.X,
                op=mybir.AluOpType.max, apply_absolute_value=True,
            )

        def w_fetch(oc, kb):
            w_tile = wp.tile([P, KB, OC], BF16, tag="w_tile")
            w_eng = nc.sync if (oc * NKB + kb) % 2 == 0 else nc.scalar
            w_eng.dma_start(
                out=w_tile,
                in_=wt_ext[
                    ds(kb * KB * P, KB * P), ds(oc * OC, OC)
                ].rearrange("(s p) o -> p s o", p=P),
            )
            return w_tile

        def matmul_half(h):
            t0 = h * TTH
            for oc in range(NOC):
                ps = [psum.tile([P, OC], F32, tag="ps", name=f"ps_{h}_{oc}_{i}")
                      for i in range(TTH)]
                for kb in range(NKB):
                    w_tile = w_fetch(oc, kb)
                    for s in range(KB):
                        k = kb * KB + s
                        for i in range(TTH):
                            nc.tensor.matmul(
                                ps[i],
                                qt_tiles[k][:, ds((t0 + i) * P, P)],
                                w_tile[:, s, :],
                                start=(k == 0),
                                stop=(k == KT - 1),
                            )
                for i in range(TTH):
                    evac(h, oc, ps, i)
                if h == 0:
                    # half-B pass 1 interleaved into the o-chunk loop: loads
                    # run ~2 chunks ahead of their quant chain, and the quant
                    # (whose PE-transposes the scheduler slots into the PE
                    # stream near this priority point) trails 2 chunks behind,
                    # so a late x arrival can never head-of-line-stall the
                    # matmul stream.
                    t_next = TTH + oc + 1
                    if t_next < TT:
                        h1_loads[t_next] = pass1_load(t_next)
                    if TTH + oc < TT:
                        pass1_quant(TTH + oc, h1_loads.pop(TTH + oc))

        def requant_half(h):
            t0 = h * TTH
            OH = O // 4
            for i in range(TTH):
                t = t0 + i
                am = stat.tile([P, 1], F32, tag="am_u")
                nc.vector.tensor_reduce(
                    out=am, in_=amp_tiles[t], axis=mybir.AxisListType.X,
                    op=mybir.AluOpType.max,
                )
                # s_y = max(s_x * absmax_u, EPS) * (1/127)
                sy = stat.tile([P, 1], F32, tag="sy")
                nc.vector.tensor_scalar(
                    out=sy, in0=am, scalar1=sx_tiles[t], scalar2=EPS,
                    op0=mybir.AluOpType.mult, op1=mybir.AluOpType.max,
                )
                nc.vector.tensor_scalar(
                    out=sy, in0=sy, scalar1=INV_QMAX, scalar2=None,
                    op0=mybir.AluOpType.mult,
                )
                rinvy = stat.tile([P, 1], F32, tag="rinv_y")
                nc.vector.reciprocal(out=rinvy, in_=sy)
                # f = s_x * rinv_y : y*rinv_y == u*f
                f = stat.tile([P, 1], F32, tag="f")
                nc.vector.tensor_scalar(
                    out=f, in0=rinvy, scalar1=sx_tiles[t], scalar2=None,
                    op0=mybir.AluOpType.mult,
                )
                for c in range(4):
                    y_half = yp.tile([P, OH], F32, tag="y_half")
                    # r = u*f + MAGIC (gpsimd), y_q = (r - MAGIC)*s_y (vector)
                    nc.gpsimd.tensor_scalar(
                        out=y_half, in0=u_tiles[i][:, ds(c * OH, OH)],
                        scalar1=f, scalar2=MAGIC,
                        op0=mybir.AluOpType.mult, op1=mybir.AluOpType.add,
                    )
                    nc.vector.tensor_scalar(
                        out=y_half, in0=y_half, scalar1=MAGIC, scalar2=sy,
                        op0=mybir.AluOpType.subtract, op1=mybir.AluOpType.mult,
                    )
                    # half-A stores ride SWDGE (W still streams on the HWDGE
                    # rings then); half-B stores split across the two HWDGE
                    # rings, which are idle once the last W block is in
                    if h == 0:
                        out_eng = nc.gpsimd
                    else:
                        out_eng = nc.sync if c % 2 == 0 else nc.scalar
                    out_eng.dma_start(
                        out=out_ext[ds(t * P, P), ds(c * OH, OH)], in_=y_half
                    )

        for h in range(2):
            matmul_half(h)
            requant_half(h)

    nc.compile()
    return nc


_NC_CACHE = None


def _get_nc():
    global _NC_CACHE
    if _NC_CACHE is None:
        _NC_CACHE = build()
    return _NC_CACHE


def _run(x, W, b, trace=False):
    nc = _get_nc()
    x2d = np.ascontiguousarray(np.asarray(x, dtype=np.float32).reshape(-1, K))
    wt = np.ascontiguousarray(np.asarray(W, dtype=np.float32).T).astype(
        ml_dtypes.bfloat16
    )
    bf = np.ascontiguousarray(np.asarray(b, dtype=np.float32))
    in_maps = [
        {"x": np.ascontiguousarray(x2d[i * T:(i + 1) * T]), "wt": wt, "b": bf}
        for i in range(N_CORES)
    ]
    res = run_bass_kernel_spmd(nc, in_maps, list(range(N_CORES)), trace=trace)
    out = np.concatenate([res.results[i]["out"] for i in range(N_CORES)], axis=0)
    return out, res


def kernel(x, W, b):
    out, _ = _run(x, W, b, trace=False)
    return out.reshape(np.asarray(x).shape[:-1] + (O,)).astype(np.float32)
